# revision 10
# baseline (speedup 1.0000x reference)
"""Multi-head attention Trainium2 kernel (8 NeuronCores).

Problem: B=2, S=4096, D=512, H=8 heads of dim 64.
Reference returns (output [B,S,D], attn_weights [B,H,S,S]).

Sharding: data-parallel over B (4 cores per batch) x tensor-parallel over
head-pairs (2 heads per core). Each core computes its 2 heads' full
attention matrix ([2,S,S] fp32, the dominant ~134MB HBM write) plus a
partial output projection; the host sums the 4 partials per batch.

Device algorithm per core (all matmuls float32r ~= tf32 precision):
  - QT/KT [128,S] and V [S,128] projections from host-pre-transposed X.
  - Pass 1 (per head, per 1024-wide q-chunk): scores^T tiles [128k,1024q]
    on PE, exp(s/8) on ScalarE (transient), AV matmul with a ones-augmented
    V so PSUM row 64 accumulates the softmax denominators.
  - Stats: denominators are transposed to [128q, col] via tiny rank-1
    matmuls; reciprocal (DVE) + Log (ScalarE) give ln(1/sum) per q.
  - Pass 2 (per head, per 128-q block): scores [q,k] on PE, ScalarE writes
    exp(s/8 + ln(1/sum)) -- the normalized softmax -- straight to the
    attention output buffer, DMA'd to HBM in natural [q,k] layout.
  - Epilogue: out_partial[q,512] = sum_h (ctx_h^T)^T @ w_o_rows_h scaled
    by recip_h[q] per partition.
"""

import numpy as np

import concourse.bass as bass
import concourse.mybir as mybir
import concourse.tile as tile
from concourse import bacc
from concourse.bass_utils import run_bass_kernel_spmd

F32 = mybir.dt.float32
F32R = mybir.dt.float32r

D_MODEL = 512
N_HEADS = 8
HEAD_DIM = 64
B = 2
S_FULL = 4096
N_CORES = 8
HP = 2  # heads per core


def build_kernel(S=S_FULL):
    """Build the per-core Bass kernel. Same program on all 8 cores."""
    assert S % 1024 == 0 or S in (256, 512)
    QCW = 1024 if S % 1024 == 0 else S  # q/k chunk width for pass1/pass2
    KB = S // 128          # number of 128-row k blocks
    NQC = S // QCW         # number of q chunks (pass 1)
    QB = S // 128          # number of 128-row q blocks (pass 2)
    NKC = S // QCW         # number of k chunks per q block (pass 2)
    CPQ = QCW // 128       # sumsT columns produced per q chunk

    nc = bacc.Bacc("TRN2")

    # ---- DRAM I/O ----
    xqt = nc.dram_tensor("xqt", [D_MODEL, S], F32R, kind="ExternalInput")
    xkt = nc.dram_tensor("xkt", [D_MODEL, S], F32R, kind="ExternalInput")
    xvt = nc.dram_tensor("xvt", [D_MODEL, S], F32R, kind="ExternalInput")
    # [513, 128]: rows 0..511 = W[rows,:].T for this head pair, row 512 = bias
    wqt = nc.dram_tensor("wqt", [D_MODEL + 1, 128], F32R, kind="ExternalInput")
    wkt = nc.dram_tensor("wkt", [D_MODEL + 1, 128], F32R, kind="ExternalInput")
    wvt = nc.dram_tensor("wvt", [D_MODEL + 1, 128], F32R, kind="ExternalInput")
    # w_o columns for this head pair, transposed: [128, 512]
    wot = nc.dram_tensor("wot", [128, D_MODEL], F32R, kind="ExternalInput")

    attn = nc.dram_tensor("attn", [HP, S, S], F32, kind="ExternalOutput")
    outp = nc.dram_tensor("outp", [S, D_MODEL], F32, kind="ExternalOutput")

    KIN = D_MODEL // 128   # 4 input-dim tiles

    with tile.TileContext(nc) as tc:
        with (
            tc.tile_pool(name="persist", bufs=1) as persist,
            tc.tile_pool(name="psA", bufs=3, space="PSUM") as psA,
            tc.tile_pool(name="psC", bufs=1, space="PSUM") as psC,
        ):
            # ---- persistent tiles ----
            QT = persist.tile([128, S], F32R)      # rows 0-63 h0, 64-127 h1
            KT = persist.tile([128, S], F32R)
            Vaug = [persist.tile([128, KB * 65], F32R, name=f"Vaug{i}")
                    for i in range(HP)]
            ctxT = [persist.tile([64, S], F32R, name=f"ctxT{i}") for i in range(HP)]
            sumsT = [persist.tile([128, QB], F32R, name=f"sumsT{i}")
                     for i in range(HP)]
            recipT = [persist.tile([128, QB], F32, name=f"recipT{i}")
                      for i in range(HP)]
            neglnT = [persist.tile([128, QB], F32, name=f"neglnT{i}")
                      for i in range(HP)]
            ones_row = persist.tile([1, 512], F32R)
            one_one = persist.tile([1, 1], F32)
            sstage = persist.tile([1, QCW], F32)

            nc.vector.memset(ones_row.bitcast(F32), 1.0)
            nc.vector.memset(one_one, 1.0)
            for i in range(HP):
                nc.vector.memset(Vaug[i].bitcast(F32), 1.0)  # ones col survives at 65k+64

            # ---- load weights ----
            wq_sb, wk_sb, wv_sb = [], [], []
            for name, dram, lst in (("q", wqt, wq_sb), ("k", wkt, wk_sb), ("v", wvt, wv_sb)):
                for i in range(KIN):
                    t = persist.tile([128, 128], F32R, name=f"w{name}{i}", tag=f"w{name}{i}")
                    nc.sync.dma_start(t[:], dram[128 * i:128 * (i + 1), :])
                    lst.append(t)
                tb = persist.tile([1, 128], F32R, name=f"w{name}b", tag=f"w{name}b")
                nc.sync.dma_start(tb[:], dram[D_MODEL:D_MODEL + 1, :])
                lst.append(tb)
            wot_h = []
            for i in range(HP):
                t = persist.tile([64, D_MODEL], F32R, name=f"wo{i}", tag=f"wo{i}")
                nc.sync.dma_start(t[:], wot[64 * i:64 * (i + 1), :])
                wot_h.append(t)

            # ---- projections ----
            with tc.tile_pool(name="xstage", bufs=1) as xstage:
                for xdram, kind in ((xqt, "q"), (xkt, "k"), (xvt, "v")):
                    xt = [xstage.tile([128, S], F32R, name=f"x{i}", tag=f"x{i}")
                          for i in range(KIN)]
                    for i in range(KIN):
                        nc.sync.dma_start(xt[i][:], xdram[128 * i:128 * (i + 1), :])
                    w = {"q": wq_sb, "k": wk_sb, "v": wv_sb}[kind]
                    if kind in ("q", "k"):
                        dst = QT if kind == "q" else KT
                        for c in range(S // 512):
                            ps = psA.tile([128, 512], F32, tag="ps")
                            for i in range(KIN):
                                nc.tensor.matmul(
                                    ps[:], w[i][:], xt[i][:, 512 * c:512 * (c + 1)],
                                    start=(i == 0), stop=False)
                            nc.tensor.matmul(
                                ps[:], w[KIN][:], ones_row[:],
                                start=False, stop=True)
                            nc.vector.tensor_copy(dst[:, 512 * c:512 * (c + 1)], ps[:])
                    else:
                        # V natural [tok, d]: lhsT = xT tile (X as weights)
                        for tb in range(KB):
                            ps = psA.tile([128, 128], F32, tag="ps")
                            for i in range(KIN):
                                nc.tensor.matmul(
                                    ps[:], xt[i][:, 128 * tb:128 * (tb + 1)], w[i][:],
                                    start=(i == 0), stop=False)
                            nc.tensor.matmul(
                                ps[:], ones_row[:, 0:128], w[KIN][:],
                                start=False, stop=True)
                            for i in range(HP):
                                nc.vector.tensor_copy(
                                    Vaug[i][:, 65 * tb:65 * tb + 64],
                                    ps[:, 64 * i:64 * (i + 1)])

            # ---- attention ----
            with (
                tc.tile_pool(name="p1exp", bufs=3) as p1exp,
                tc.tile_pool(name="attnout", bufs=3) as attnout,
                tc.tile_pool(name="opart", bufs=3) as opart,
            ):
                for h in range(HP):
                    qlo, qhi = 64 * h, 64 * (h + 1)

                    # ---- pass 1: scores^T -> exp -> AV(+sums) ----
                    for qc in range(NQC):
                        ctx_ps = psC.tile([65, QCW], F32, tag="ctx")
                        for kb in range(KB):
                            sps = psA.tile([128, QCW], F32, tag="ps")
                            for half in range(QCW // 512):
                                sl = slice(512 * half, 512 * (half + 1))
                                nc.tensor.matmul(
                                    sps[:, sl],
                                    KT[qlo:qhi, 128 * kb:128 * (kb + 1)],
                                    QT[qlo:qhi, QCW * qc + 512 * half:
                                       QCW * qc + 512 * (half + 1)],
                                    start=True, stop=True)
                            et = p1exp.tile([128, QCW], F32R, tag="exp")
                            nc.scalar.activation(
                                et[:], sps[:],
                                mybir.ActivationFunctionType.Exp, scale=0.125)
                            for half in range(QCW // 512):
                                sl = slice(512 * half, 512 * (half + 1))
                                nc.tensor.matmul(
                                    ctx_ps[:, sl],
                                    Vaug[h][:, 65 * kb:65 * (kb + 1)],
                                    et[:, sl],
                                    start=(kb == 0), stop=(kb == KB - 1))
                        # copy ctx + sums out of PSUM
                        nc.vector.tensor_copy(
                            ctxT[h][:, QCW * qc:QCW * (qc + 1)], ctx_ps[0:64, :])
                        nc.vector.tensor_copy(sstage[:], ctx_ps[64:65, :])
                        # transpose sums [1,QCW] -> sumsT columns via rank-1 mms
                        for j in range(CPQ):
                            tp = psA.tile([128, 1], F32, tag="ps")
                            nc.tensor.matmul(
                                tp[:], sstage[:, 128 * j:128 * (j + 1)], one_one[:],
                                start=True, stop=True)
                            nc.vector.tensor_copy(
                                sumsT[h][:, CPQ * qc + j:CPQ * qc + j + 1], tp[:])
                    # whole-head stats in single ops: keeps the ACT stream
                    # exp* -> Ln -> exp* (minimal act-table reloads)
                    nc.vector.reciprocal(recipT[h][:], sumsT[h][:])
                    nc.scalar.activation(
                        neglnT[h][:], recipT[h][:],
                        mybir.ActivationFunctionType.Ln)

                    # ---- pass 2: scores [q,k] -> normalized exp -> HBM ----
                    for qb in range(QB):
                        at = attnout.tile([128, S], F32, tag="attn")
                        for kc in range(NKC):
                            sps = psA.tile([128, QCW], F32, tag="ps")
                            for half in range(QCW // 512):
                                sl = slice(512 * half, 512 * (half + 1))
                                nc.tensor.matmul(
                                    sps[:, sl],
                                    QT[qlo:qhi, 128 * qb:128 * (qb + 1)],
                                    KT[qlo:qhi, QCW * kc + 512 * half:
                                       QCW * kc + 512 * (half + 1)],
                                    start=True, stop=True)
                            nc.scalar.activation(
                                at[:, QCW * kc:QCW * (kc + 1)], sps[:],
                                mybir.ActivationFunctionType.Exp,
                                bias=neglnT[h][:, qb:qb + 1], scale=0.125)
                        nc.sync.dma_start(
                            attn[h, 128 * qb:128 * (qb + 1), :], at[:])

                # ---- epilogue: output projection ----
                # accumulate GRP q-blocks into one SBUF tile -> 1 DMA each
                GRP = min(4, QB)
                outp_v = outp.rearrange("(g b) d -> g b d", b=128 * GRP)
                for qg in range(QB // GRP):
                    acc = opart.tile([128, GRP * D_MODEL], F32, tag="opacc")
                    for qi in range(GRP):
                        qb = qg * GRP + qi
                        osl = slice(D_MODEL * qi, D_MODEL * (qi + 1))
                        for h in range(HP):
                            ops = psA.tile([128, D_MODEL], F32, tag="ps")
                            nc.tensor.matmul(
                                ops[:], ctxT[h][:, 128 * qb:128 * (qb + 1)],
                                wot_h[h][:], start=True, stop=True)
                            if h == 0:
                                nc.vector.tensor_scalar_mul(
                                    acc[:, osl], ops[:], recipT[h][:, qb:qb + 1])
                            else:
                                ot = opart.tile([128, D_MODEL], F32, tag="oph")
                                nc.vector.tensor_scalar_mul(
                                    ot[:], ops[:], recipT[h][:, qb:qb + 1])
                                nc.vector.tensor_add(
                                    acc[:, osl], acc[:, osl], ot[:])
                    dst = outp_v[qg].rearrange("(b p) d -> p b d", p=128)
                    src = acc[:].rearrange("p (b d) -> p b d", d=D_MODEL)
                    nc.sync.dma_start(dst, src)

    nc.compile()
    return nc


_NC_CACHE = {}


def _get_kernel(S):
    if S not in _NC_CACHE:
        _NC_CACHE[S] = build_kernel(S)
    return _NC_CACHE[S]


def _make_in_maps(inputs):
    query = np.asarray(inputs["query"], dtype=np.float32)
    key = np.asarray(inputs["key"], dtype=np.float32)
    value = np.asarray(inputs["value"], dtype=np.float32)
    w_q, b_q = np.asarray(inputs["w_q"], np.float32), np.asarray(inputs["b_q"], np.float32)
    w_k, b_k = np.asarray(inputs["w_k"], np.float32), np.asarray(inputs["b_k"], np.float32)
    w_v, b_v = np.asarray(inputs["w_v"], np.float32), np.asarray(inputs["b_v"], np.float32)
    w_o = np.asarray(inputs["w_o"], np.float32)

    b = query.shape[0]
    xq_t = [np.ascontiguousarray(query[i].T) for i in range(b)]
    xk_t = [np.ascontiguousarray(key[i].T) for i in range(b)]
    xv_t = [np.ascontiguousarray(value[i].T) for i in range(b)]

    in_maps = []
    for c in range(N_CORES):
        bi, hp = c // 4, c % 4
        rows = slice(128 * hp, 128 * (hp + 1))
        in_maps.append({
            "xqt": xq_t[bi], "xkt": xk_t[bi], "xvt": xv_t[bi],
            "wqt": np.ascontiguousarray(
                np.concatenate([w_q[rows].T, b_q[rows][None, :]], axis=0)),
            "wkt": np.ascontiguousarray(
                np.concatenate([w_k[rows].T, b_k[rows][None, :]], axis=0)),
            "wvt": np.ascontiguousarray(
                np.concatenate([w_v[rows].T, b_v[rows][None, :]], axis=0)),
            "wot": np.ascontiguousarray(w_o[:, rows].T),
        })
    return in_maps


def kernel(query, key, value, w_q, b_q, w_k, b_k, w_v, b_v, w_o, b_o):
    inputs = {
        "query": query, "key": key, "value": value,
        "w_q": w_q, "b_q": b_q, "w_k": w_k, "b_k": b_k,
        "w_v": w_v, "b_v": b_v, "w_o": w_o, "b_o": b_o,
    }
    query = np.asarray(query, dtype=np.float32)
    b_o = np.asarray(b_o, np.float32)
    b, s, _ = query.shape
    nc = _get_kernel(s)
    in_maps = _make_in_maps(inputs)

    res = run_bass_kernel_spmd(nc, in_maps, core_ids=list(range(N_CORES)))

    attn_full = np.empty((b, N_HEADS, s, s), dtype=np.float32)
    out_full = np.zeros((b, s, D_MODEL), dtype=np.float32)
    for c in range(N_CORES):
        bi, hp = c // 4, c % 4
        attn_full[bi, 2 * hp] = res.results[c]["attn"][0]
        attn_full[bi, 2 * hp + 1] = res.results[c]["attn"][1]
        out_full[bi] += res.results[c]["outp"]
    out_full += b_o
    return out_full, attn_full
